# revision 10
# baseline (speedup 1.0000x reference)
"""Trainium2 Bass kernel for nn_LinearSelector (ragged passage scoring).

Math (per batch b):
  q_rep  = masked-mean of emb[queries[b]]            -> [D]
  p_rep  = per-passage mean of emb[docs[b]] segments -> [P, D]
  q_enc  = q_rep @ W.T + bias ; p_enc = p_rep @ W.T + bias
  scores[b, p] = q_enc . p_enc[p]   (masked by passage_lengths > 0)

Device strategy (data-parallel over batch, 4 batches/core on 8 cores):
  - One token stream per core: 32 query tokens + all real doc tokens of its
    4 batches, bucketed by vocab half (dma_gather indices are int16).
  - dma_gather pulls 512 embedding rows per call into SBUF [128, 4, D].
  - Segment sums via PE matmul with a host-built 0/1 weight matrix
    (lhsT [128 tokens, 68 segs]) accumulating into PSUM [68, D].
  - reps /= length (per-partition scalar), transpose via PE to [D, 68],
    enc = reps @ W.T + bias (W.T streamed from HBM, host-pretransposed),
    scores = rowwise dot of enc with its batch's query row (selector matmul
    + elementwise mul + free-axis reduce).
  - Host gathers the per-core [68] score vectors back to [32, 16].
"""

import sys

if "/opt/trn_rl_repo" not in sys.path:
    sys.path.insert(0, "/opt/trn_rl_repo")

import numpy as np
import ml_dtypes

B, LQ, DLEN, NPASS = 32, 32, 4096, 16
V, D, H = 50257, 2048, 2048
NCORES = 8
BPC = B // NCORES            # batches per core
SEGS = NPASS + 1             # query + 16 passages
M = BPC * SEGS               # 68 segments per core
C = 512                      # tokens per dma_gather call
SUB = C // 128               # 128-token subtiles per call
SPLIT = 32768                # int16 vocab bucket split
DTYPE = "bf16"                # "fp8" | "bf16" | "f32" for emb/gather/segment-sum
MPAD = 80                    # lhsT seg-dim padded so DoubleRow AP step % 16 == 0

_prog_cache: dict = {}


def _np_dt():
    return {"fp8": ml_dtypes.float8_e4m3, "bf16": ml_dtypes.bfloat16, "f32": np.float32}[DTYPE]


def _np_wdt():
    return ml_dtypes.bfloat16 if DTYPE != "f32" else np.float32


def _build_program(TA: int, TB: int):
    import concourse.tile as tile
    from concourse import bacc, mybir

    dt = {"fp8": mybir.dt.float8e4, "bf16": mybir.dt.bfloat16, "f32": mybir.dt.float32}[DTYPE]
    wdt = mybir.dt.bfloat16 if DTYPE != "f32" else mybir.dt.float32
    f32 = mybir.dt.float32
    fp8 = DTYPE == "fp8"
    mseg = 128  # lhsT seg-dim padded to 128 so LDWEIGHTS uses Fast Weight Load
    T = TA + TB
    TSUB = T * SUB

    nc = bacc.Bacc(
        "TRN2",
        target_bir_lowering=False,
        debug=False,
        enable_asserts=False,
        num_swdge_queues=4,
        dynamic_dma_scratch_size=32768,
    )
    emb_t = nc.dram_tensor("emb", [V, D], dt, kind="ExternalInput")
    wt_t = nc.dram_tensor("wt", [128, 16, H], wdt, kind="ExternalInput")
    wts_t = nc.dram_tensor("wts", [128, TSUB, mseg], dt, kind="ExternalInput")
    idx_t = nc.dram_tensor("idx", [128, T * C // 16], mybir.dt.int16, kind="ExternalInput")
    recip_t = nc.dram_tensor("recip", [M, 1], f32, kind="ExternalInput")
    beta_t = nc.dram_tensor("beta", [1, H], f32, kind="ExternalInput")
    iden_t = nc.dram_tensor("identity", [128, 128], f32, kind="ExternalInput")
    ones_t = nc.dram_tensor("ones1", [1, M], f32, kind="ExternalInput")
    selt_t = nc.dram_tensor("selt", [M, M], f32, kind="ExternalInput")
    scores_t = nc.dram_tensor("scores", [M, 1], f32, kind="ExternalOutput")

    with tile.TileContext(nc) as tc:
        with (
            tc.tile_pool(name="const", bufs=1) as const,
            tc.tile_pool(name="gather", bufs=4) as gather,
            tc.tile_pool(name="wtp", bufs=2) as wtp,
        ):
            idx_sb = const.tile([128, T * C // 16], mybir.dt.int16)
            wts_sb = const.tile([128, TSUB, mseg], dt)

            reps_sb = const.tile([128, D], f32)
            # rows >= M are only consumed by the PE transpose (whose M-trailing
            # output columns are discarded); zero them from partition 64 (DVE
            # base-partition must be a multiple of 32) — rows 64..M-1 are
            # rewritten by the divide below before any read.
            nc.vector.memset(reps_sb[64:128, :], 0.0)

            # --- gather + segment-sum accumulation ---
            with tc.tile_pool(name="psum_reps", bufs=1, space="PSUM") as pr:
                reps_ps = [pr.tile([mseg, 512], f32, tag=f"r{n}", name=f"reps_ps{n}") for n in range(4)]
                for g in range(T):
                    src = emb_t[0:SPLIT, :] if g < TA else emb_t[SPLIT:V, :]
                    # per-call metadata slices: keeps the first gather/matmuls
                    # from waiting on one monolithic const DMA at startup
                    nc.sync.dma_start(
                        idx_sb[:, g * (C // 16) : (g + 1) * (C // 16)],
                        idx_t[:, g * (C // 16) : (g + 1) * (C // 16)],
                    )
                    nc.sync.dma_start(
                        wts_sb[:, g * SUB : (g + 1) * SUB, :],
                        wts_t[:, g * SUB : (g + 1) * SUB, :],
                    )
                    buf = gather.tile([128, SUB, D], dt, tag="gbuf", name=f"gbuf{g}")
                    nc.gpsimd.dma_gather(
                        buf[:],
                        src,
                        idx_sb[:, g * (C // 16) : (g + 1) * (C // 16)],
                        C,
                        C,
                        D,
                        queue_num=g % 4,
                    )
                    if fp8:
                        for s in range(0, SUB, 2):
                            tt = g * SUB + s
                            for n in range(4):
                                nc.tensor.matmul(
                                    reps_ps[n][:, :],
                                    wts_sb[:, tt : tt + 2, :],
                                    buf[:, s : s + 2, n * 512 : (n + 1) * 512],
                                    start=(tt == 0),
                                    stop=(tt == TSUB - 2),
                                    perf_mode=mybir.MatmulPerfMode.DoubleRow,
                                )
                    else:
                        for s in range(SUB):
                            tt = g * SUB + s
                            for n in range(4):
                                nc.tensor.matmul(
                                    reps_ps[n][:, :],
                                    wts_sb[:, tt, :],
                                    buf[:, s, n * 512 : (n + 1) * 512],
                                    start=(tt == 0),
                                    stop=(tt == TSUB - 1),
                                )
                # reps = segment sums / lengths
                recip_sb = const.tile([M, 1], f32)
                nc.sync.dma_start(recip_sb[:], recip_t[:, :])
                for n in range(4):
                    nc.vector.tensor_scalar_mul(
                        reps_sb[:M, n * 512 : (n + 1) * 512],
                        reps_ps[n][:M, :],
                        recip_sb[:, :],
                    )

            # --- transpose reps -> repsT tiles [128, 16, M] ---
            iden_sb = const.tile([128, 128], f32)
            nc.sync.dma_start(iden_sb[:], iden_t[:, :])
            beta_sb = const.tile([1, H], f32)
            nc.sync.dma_start(beta_sb[:], beta_t[:, :])
            ones_sb = const.tile([1, M], f32)
            nc.sync.dma_start(ones_sb[:], ones_t[:, :])
            selt_sb = const.tile([M, M], f32)
            nc.sync.dma_start(selt_sb[:], selt_t[:, :])
            repsT = const.tile([128, 16, M], wdt)
            with tc.tile_pool(name="psum_tr", bufs=2, space="PSUM") as pt:
                for k in range(16):
                    trp = pt.tile([128, 128], f32, tag="tr", name=f"trp{k}")
                    nc.tensor.transpose(
                        trp[:], reps_sb[:, k * 128 : (k + 1) * 128], iden_sb[:]
                    )
                    nc.vector.tensor_copy(repsT[:, k, :], trp[:, :M])

            # --- enc = bias + reps @ W.T (W.T streamed) ---
            enc_sb = const.tile([M, H], f32)
            with tc.tile_pool(name="psum_enc", bufs=1, space="PSUM") as pe:
                enc_ps = [pe.tile([M, 512], f32, tag=f"e{n}", name=f"enc_ps{n}") for n in range(4)]
                for n in range(4):
                    nc.tensor.matmul(
                        enc_ps[n][:, :],
                        ones_sb[:, :],
                        beta_sb[:, n * 512 : (n + 1) * 512],
                        start=True,
                        stop=False,
                    )
                for kg in range(4):
                    slab = wtp.tile([128, 4, H], wdt, tag="wslab", name=f"wslab{kg}")
                    nc.sync.dma_start(slab[:], wt_t[:, kg * 4 : (kg + 1) * 4, :])
                    for k2 in range(4):
                        k = kg * 4 + k2
                        for n in range(4):
                            nc.tensor.matmul(
                                enc_ps[n][:, :],
                                repsT[:, k, :],
                                slab[:, k2, n * 512 : (n + 1) * 512],
                                start=False,
                                stop=(k == 15),
                            )
                for n in range(4):
                    nc.vector.tensor_copy(
                        enc_sb[:, n * 512 : (n + 1) * 512], enc_ps[n][:, :]
                    )

            # --- scores: row r of enc dotted with its batch's query row ---
            prod = const.tile([M, H], f32)
            with tc.tile_pool(name="psum_qx", bufs=1, space="PSUM") as pq:
                for n in range(4):
                    qx_ps = pq.tile([M, 512], f32, tag=f"q{n}", name=f"qx_ps{n}")
                    nc.tensor.matmul(
                        qx_ps[:, :],
                        selt_sb[:, :],
                        enc_sb[:, n * 512 : (n + 1) * 512],
                        start=True,
                        stop=True,
                    )
                    nc.vector.tensor_mul(
                        prod[:, n * 512 : (n + 1) * 512],
                        enc_sb[:, n * 512 : (n + 1) * 512],
                        qx_ps[:, :],
                    )
            sc = const.tile([M, 1], f32)
            nc.vector.reduce_sum(sc[:], prod[:], axis=mybir.AxisListType.X)
            nc.sync.dma_start(scores_t[:, :], sc[:])

    nc.compile()
    return nc


def _plan(queries, qlen, docs, plen):
    """Assign batches to cores (greedy-balanced) and build per-core token
    streams: (ids, segs) with seg = 17*local_b + (0 for query, 1+p for
    passage p), seg == -1 for masked/padding tokens."""
    cum = np.cumsum(plen, axis=1)
    total = cum[:, -1]
    counts = LQ + total
    order = np.argsort(-counts, kind="stable")
    loads = [0] * NCORES
    assign: list[list[int]] = [[] for _ in range(NCORES)]
    for b in order:
        cands = [c for c in range(NCORES) if len(assign[c]) < BPC]
        c = min(cands, key=lambda c: loads[c])
        assign[c].append(int(b))
        loads[c] += int(counts[b])

    streams = []
    for c in range(NCORES):
        ids_l, seg_l = [], []
        for j, b in enumerate(assign[c]):
            ids_l.append(queries[b].astype(np.int64))
            seg_l.append(np.where(np.arange(LQ) < qlen[b], SEGS * j, -1))
            nb = int(total[b])
            ids_l.append(docs[b, :nb].astype(np.int64))
            pid = np.searchsorted(cum[b], np.arange(nb), side="right")
            seg_l.append(SEGS * j + 1 + pid)
        streams.append((np.concatenate(ids_l), np.concatenate(seg_l)))
    return assign, streams


def kernel(**inputs) -> np.ndarray:
    queries = np.asarray(inputs["queries"]).astype(np.int64)
    qlen = np.asarray(inputs["query_lengths"]).astype(np.int64)
    docs = np.asarray(inputs["docs"]).astype(np.int64)
    plen = np.asarray(inputs["passage_lengths"]).astype(np.int64)
    emb = np.asarray(inputs["emb"], dtype=np.float32)
    W = np.asarray(inputs["W"], dtype=np.float32)
    bias = np.asarray(inputs["b"], dtype=np.float32)

    outputs, _ = _run(queries, qlen, docs, plen, emb, W, bias, trace=False)
    return outputs


def _run(queries, qlen, docs, plen, emb, W, bias, trace=False):
    TA, TB, assign, in_maps = _prepare(queries, qlen, docs, plen, emb, W, bias)

    key = (TA, TB, DTYPE)
    if key not in _prog_cache:
        _prog_cache[key] = _build_program(TA, TB)
    nc = _prog_cache[key]

    from concourse.bass_utils import run_bass_kernel_spmd

    res = run_bass_kernel_spmd(
        nc, in_maps, core_ids=list(range(NCORES)), trace=trace
    )

    out = np.zeros((B, NPASS), np.float32)
    for c in range(NCORES):
        vec = np.asarray(res.results[c]["scores"]).reshape(M)
        for j, b in enumerate(assign[c]):
            out[b, :] = vec[SEGS * j + 1 : SEGS * (j + 1)]
    out *= (plen > 0).astype(np.float32)
    return out, res


def _prepare(queries, qlen, docs, plen, emb, W, bias):
    """Host-side planning: batch->core assignment, token streams, and the 8
    per-core input maps (all index/weight metadata precomputed)."""
    np_dt = _np_dt()
    np_wdt = _np_wdt()
    mseg = 128
    assign, streams = _plan(queries, qlen, docs, plen)

    # Dedup repeated token ids within a core's stream: gather each distinct
    # row once; its weight-matrix entry becomes the occurrence count per
    # segment (small integers -> exact in bf16/fp8).
    dstreams = []
    for ids, segs in streams:
        keep = segs >= 0
        ids, segs = ids[keep], segs[keep]
        uids, inv = np.unique(ids, return_inverse=True)
        wcnt = np.zeros((len(uids), M), np.float32)
        np.add.at(wcnt, (inv, segs), 1.0)
        dstreams.append((uids, wcnt))

    buckets = []
    for uids, wcnt in dstreams:
        mA = uids < SPLIT
        buckets.append(((uids[mA], wcnt[mA]), (uids[~mA] - SPLIT, wcnt[~mA])))
    TA = max(1, max(-(-len(a[0]) // C) for a, _ in buckets))
    TB = max(1, max(-(-len(b[0]) // C) for _, b in buckets))
    T = TA + TB
    TSUB = T * SUB

    # fp8 needs a power-of-two scale to lift emb out of the subnormal range;
    # it is divided back out via the per-segment reciprocals (f32).
    scale = 1.0
    if DTYPE == "fp8":
        amax = float(np.abs(emb).max())
        scale = float(2.0 ** np.floor(np.log2(400.0 / amax)))

    emb_c = np.ascontiguousarray((emb * scale).astype(np_dt) if scale != 1.0 else emb.astype(np_dt))
    wt_host = np.ascontiguousarray(W.T.reshape(16, 128, H).transpose(1, 0, 2).astype(np_wdt))
    iden = np.eye(128, dtype=np.float32)
    ones1 = np.ones((1, M), np.float32)
    selt = np.zeros((M, M), np.float32)
    for j in range(BPC):
        selt[SEGS * j, SEGS * j : SEGS * (j + 1)] = 1.0
    beta = bias.reshape(1, H).astype(np.float32)

    in_maps = []
    for c in range(NCORES):
        (aids, awc), (bids, bwc) = buckets[c]
        sid = np.zeros(T * C, np.int64)
        w = np.zeros((T * C, mseg), np.float32)
        sid[: len(aids)] = aids
        w[: len(aids), :M] = awc
        sid[TA * C : TA * C + len(bids)] = bids
        w[TA * C : TA * C + len(bids), :M] = bwc

        i16 = sid.astype(np.int16)
        idx_host = np.ascontiguousarray(np.tile(i16.reshape(-1, 16).T, (8, 1)))

        wts_host = np.ascontiguousarray(
            w.reshape(TSUB, 128, mseg).transpose(1, 0, 2).astype(np_dt)
        )

        recip = np.zeros((M, 1), np.float32)
        for j, b in enumerate(assign[c]):
            recip[SEGS * j, 0] = 1.0 / (float(qlen[b]) * scale)
            for p in range(NPASS):
                recip[SEGS * j + 1 + p, 0] = 1.0 / (float(max(int(plen[b, p]), 1)) * scale)

        in_maps.append(
            dict(
                emb=emb_c,
                wt=wt_host,
                wts=wts_host,
                idx=idx_host,
                recip=recip,
                beta=beta,
                identity=iden,
                ones1=ones1,
                selt=selt,
            )
        )
    return TA, TB, assign, in_maps


# revision 11
# speedup vs baseline: 1.0191x; 1.0191x over previous
"""Trainium2 Bass kernel for nn_LinearSelector (ragged passage scoring).

Math (per batch b):
  q_rep  = masked-mean of emb[queries[b]]            -> [D]
  p_rep  = per-passage mean of emb[docs[b]] segments -> [P, D]
  q_enc  = q_rep @ W.T + bias ; p_enc = p_rep @ W.T + bias
  scores[b, p] = q_enc . p_enc[p]   (masked by passage_lengths > 0)

Device strategy (data-parallel over batch, 4 batches/core on 8 cores):
  - One token stream per core: 32 query tokens + all real doc tokens of its
    4 batches, bucketed by vocab half (dma_gather indices are int16).
  - dma_gather pulls 512 embedding rows per call into SBUF [128, 4, D].
  - Segment sums via PE matmul with a host-built 0/1 weight matrix
    (lhsT [128 tokens, 68 segs]) accumulating into PSUM [68, D].
  - reps /= length (per-partition scalar), transpose via PE to [D, 68],
    enc = reps @ W.T + bias (W.T streamed from HBM, host-pretransposed),
    scores = rowwise dot of enc with its batch's query row (selector matmul
    + elementwise mul + free-axis reduce).
  - Host gathers the per-core [68] score vectors back to [32, 16].
"""

import sys

if "/opt/trn_rl_repo" not in sys.path:
    sys.path.insert(0, "/opt/trn_rl_repo")

import numpy as np
import ml_dtypes

B, LQ, DLEN, NPASS = 32, 32, 4096, 16
V, D, H = 50257, 2048, 2048
NCORES = 8
BPC = B // NCORES            # batches per core
SEGS = NPASS + 1             # query + 16 passages
M = BPC * SEGS               # 68 segments per core
C = 512                      # tokens per dma_gather call
SUB = C // 128               # 128-token subtiles per call
SPLIT = 32768                # int16 vocab bucket split
DTYPE = "bf16"                # "fp8" | "bf16" | "f32" for emb/gather/segment-sum
MPAD = 80                    # lhsT seg-dim padded so DoubleRow AP step % 16 == 0

_prog_cache: dict = {}


def _np_dt():
    return {"fp8": ml_dtypes.float8_e4m3, "bf16": ml_dtypes.bfloat16, "f32": np.float32}[DTYPE]


def _np_wdt():
    return ml_dtypes.bfloat16 if DTYPE != "f32" else np.float32


def _build_program(TA: int, TB: int):
    import concourse.tile as tile
    from concourse import bacc, mybir

    dt = {"fp8": mybir.dt.float8e4, "bf16": mybir.dt.bfloat16, "f32": mybir.dt.float32}[DTYPE]
    wdt = mybir.dt.bfloat16 if DTYPE != "f32" else mybir.dt.float32
    f32 = mybir.dt.float32
    fp8 = DTYPE == "fp8"
    mseg = MPAD if fp8 else M
    T = TA + TB
    TSUB = T * SUB

    nc = bacc.Bacc(
        "TRN2",
        target_bir_lowering=False,
        debug=False,
        enable_asserts=False,
        num_swdge_queues=4,
        dynamic_dma_scratch_size=32768,
    )
    emb_t = nc.dram_tensor("emb", [V, D], dt, kind="ExternalInput")
    wt_t = nc.dram_tensor("wt", [128, 16, H], wdt, kind="ExternalInput")
    wts_t = nc.dram_tensor("wts", [128, TSUB, mseg], dt, kind="ExternalInput")
    idx_t = nc.dram_tensor("idx", [128, T * C // 16], mybir.dt.int16, kind="ExternalInput")
    recip_t = nc.dram_tensor("recip", [M, 1], f32, kind="ExternalInput")
    beta_t = nc.dram_tensor("beta", [1, H], f32, kind="ExternalInput")
    iden_t = nc.dram_tensor("identity", [128, 128], f32, kind="ExternalInput")
    ones_t = nc.dram_tensor("ones1", [1, M], f32, kind="ExternalInput")
    selt_t = nc.dram_tensor("selt", [M, M], f32, kind="ExternalInput")
    scores_t = nc.dram_tensor("scores", [M, 1], f32, kind="ExternalOutput")

    with tile.TileContext(nc) as tc:
        with (
            tc.tile_pool(name="const", bufs=1) as const,
            tc.tile_pool(name="gather", bufs=4) as gather,
            tc.tile_pool(name="wtp", bufs=2) as wtp,
        ):
            idx_sb = const.tile([128, T * C // 16], mybir.dt.int16)
            wts_sb = const.tile([128, TSUB, mseg], dt)

            reps_sb = const.tile([128, D], f32)
            # rows >= M are only consumed by the PE transpose (whose M-trailing
            # output columns are discarded); zero them from partition 64 (DVE
            # base-partition must be a multiple of 32) — rows 64..M-1 are
            # rewritten by the divide below before any read.
            nc.vector.memset(reps_sb[64:128, :], 0.0)

            # --- gather + segment-sum accumulation ---
            with tc.tile_pool(name="psum_reps", bufs=1, space="PSUM") as pr:
                reps_ps = [pr.tile([mseg, 512], f32, tag=f"r{n}", name=f"reps_ps{n}") for n in range(4)]
                for g in range(T):
                    src = emb_t[0:SPLIT, :] if g < TA else emb_t[SPLIT:V, :]
                    # per-call metadata slices: keeps the first gather/matmuls
                    # from waiting on one monolithic const DMA at startup
                    nc.sync.dma_start(
                        idx_sb[:, g * (C // 16) : (g + 1) * (C // 16)],
                        idx_t[:, g * (C // 16) : (g + 1) * (C // 16)],
                    )
                    nc.sync.dma_start(
                        wts_sb[:, g * SUB : (g + 1) * SUB, :],
                        wts_t[:, g * SUB : (g + 1) * SUB, :],
                    )
                    buf = gather.tile([128, SUB, D], dt, tag="gbuf", name=f"gbuf{g}")
                    nc.gpsimd.dma_gather(
                        buf[:],
                        src,
                        idx_sb[:, g * (C // 16) : (g + 1) * (C // 16)],
                        C,
                        C,
                        D,
                        queue_num=g % 4,
                    )
                    if fp8:
                        for s in range(0, SUB, 2):
                            tt = g * SUB + s
                            for n in range(4):
                                nc.tensor.matmul(
                                    reps_ps[n][:, :],
                                    wts_sb[:, tt : tt + 2, :],
                                    buf[:, s : s + 2, n * 512 : (n + 1) * 512],
                                    start=(tt == 0),
                                    stop=(tt == TSUB - 2),
                                    perf_mode=mybir.MatmulPerfMode.DoubleRow,
                                )
                    else:
                        for s in range(SUB):
                            tt = g * SUB + s
                            for n in range(4):
                                nc.tensor.matmul(
                                    reps_ps[n][:, :],
                                    wts_sb[:, tt, :],
                                    buf[:, s, n * 512 : (n + 1) * 512],
                                    start=(tt == 0),
                                    stop=(tt == TSUB - 1),
                                )
                # reps = segment sums / lengths
                recip_sb = const.tile([M, 1], f32)
                nc.sync.dma_start(recip_sb[:], recip_t[:, :])
                for n in range(4):
                    nc.vector.tensor_scalar_mul(
                        reps_sb[:M, n * 512 : (n + 1) * 512],
                        reps_ps[n][:M, :],
                        recip_sb[:, :],
                    )

            # --- transpose reps -> repsT tiles [128, 16, M] ---
            iden_sb = const.tile([128, 128], f32)
            nc.sync.dma_start(iden_sb[:], iden_t[:, :])
            beta_sb = const.tile([1, H], f32)
            nc.sync.dma_start(beta_sb[:], beta_t[:, :])
            ones_sb = const.tile([1, M], f32)
            nc.sync.dma_start(ones_sb[:], ones_t[:, :])
            selt_sb = const.tile([M, M], f32)
            nc.sync.dma_start(selt_sb[:], selt_t[:, :])
            repsT = const.tile([128, 16, M], wdt)
            with tc.tile_pool(name="psum_tr", bufs=2, space="PSUM") as pt:
                for k in range(16):
                    trp = pt.tile([128, 128], f32, tag="tr", name=f"trp{k}")
                    nc.tensor.transpose(
                        trp[:], reps_sb[:, k * 128 : (k + 1) * 128], iden_sb[:]
                    )
                    nc.vector.tensor_copy(repsT[:, k, :], trp[:, :M])

            # --- enc = bias + reps @ W.T (W.T streamed) ---
            enc_sb = const.tile([M, H], f32)
            with tc.tile_pool(name="psum_enc", bufs=1, space="PSUM") as pe:
                enc_ps = [pe.tile([M, 512], f32, tag=f"e{n}", name=f"enc_ps{n}") for n in range(4)]
                for n in range(4):
                    nc.tensor.matmul(
                        enc_ps[n][:, :],
                        ones_sb[:, :],
                        beta_sb[:, n * 512 : (n + 1) * 512],
                        start=True,
                        stop=False,
                    )
                slabs = []
                for kg in range(4):
                    slab = wtp.tile([128, 4, H], wdt, tag="wslab", name=f"wslab{kg}")
                    nc.scalar.dma_start(slab[:], wt_t[:, kg * 4 : (kg + 1) * 4, :])
                    slabs.append(slab)
                for kg in range(4):
                    slab = slabs[kg]
                    for k2 in range(4):
                        k = kg * 4 + k2
                        for n in range(4):
                            nc.tensor.matmul(
                                enc_ps[n][:, :],
                                repsT[:, k, :],
                                slab[:, k2, n * 512 : (n + 1) * 512],
                                start=False,
                                stop=(k == 15),
                            )
                for n in range(4):
                    nc.vector.tensor_copy(
                        enc_sb[:, n * 512 : (n + 1) * 512], enc_ps[n][:, :]
                    )

            # --- scores: row r of enc dotted with its batch's query row ---
            prod = const.tile([M, H], f32)
            with tc.tile_pool(name="psum_qx", bufs=1, space="PSUM") as pq:
                for n in range(4):
                    qx_ps = pq.tile([M, 512], f32, tag=f"q{n}", name=f"qx_ps{n}")
                    nc.tensor.matmul(
                        qx_ps[:, :],
                        selt_sb[:, :],
                        enc_sb[:, n * 512 : (n + 1) * 512],
                        start=True,
                        stop=True,
                    )
                    nc.vector.tensor_mul(
                        prod[:, n * 512 : (n + 1) * 512],
                        enc_sb[:, n * 512 : (n + 1) * 512],
                        qx_ps[:, :],
                    )
            sc = const.tile([M, 1], f32)
            nc.vector.reduce_sum(sc[:], prod[:], axis=mybir.AxisListType.X)
            nc.sync.dma_start(scores_t[:, :], sc[:])

    nc.compile()
    return nc


def _plan(queries, qlen, docs, plen):
    """Assign batches to cores (greedy-balanced) and build per-core token
    streams: (ids, segs) with seg = 17*local_b + (0 for query, 1+p for
    passage p), seg == -1 for masked/padding tokens."""
    cum = np.cumsum(plen, axis=1)
    total = cum[:, -1]
    counts = LQ + total
    order = np.argsort(-counts, kind="stable")
    loads = [0] * NCORES
    assign: list[list[int]] = [[] for _ in range(NCORES)]
    for b in order:
        cands = [c for c in range(NCORES) if len(assign[c]) < BPC]
        c = min(cands, key=lambda c: loads[c])
        assign[c].append(int(b))
        loads[c] += int(counts[b])

    streams = []
    for c in range(NCORES):
        ids_l, seg_l = [], []
        for j, b in enumerate(assign[c]):
            ids_l.append(queries[b].astype(np.int64))
            seg_l.append(np.where(np.arange(LQ) < qlen[b], SEGS * j, -1))
            nb = int(total[b])
            ids_l.append(docs[b, :nb].astype(np.int64))
            pid = np.searchsorted(cum[b], np.arange(nb), side="right")
            seg_l.append(SEGS * j + 1 + pid)
        streams.append((np.concatenate(ids_l), np.concatenate(seg_l)))
    return assign, streams


def kernel(**inputs) -> np.ndarray:
    queries = np.asarray(inputs["queries"]).astype(np.int64)
    qlen = np.asarray(inputs["query_lengths"]).astype(np.int64)
    docs = np.asarray(inputs["docs"]).astype(np.int64)
    plen = np.asarray(inputs["passage_lengths"]).astype(np.int64)
    emb = np.asarray(inputs["emb"], dtype=np.float32)
    W = np.asarray(inputs["W"], dtype=np.float32)
    bias = np.asarray(inputs["b"], dtype=np.float32)

    outputs, _ = _run(queries, qlen, docs, plen, emb, W, bias, trace=False)
    return outputs


def _run(queries, qlen, docs, plen, emb, W, bias, trace=False):
    TA, TB, assign, in_maps = _prepare(queries, qlen, docs, plen, emb, W, bias)

    key = (TA, TB, DTYPE)
    if key not in _prog_cache:
        _prog_cache[key] = _build_program(TA, TB)
    nc = _prog_cache[key]

    from concourse.bass_utils import run_bass_kernel_spmd

    res = run_bass_kernel_spmd(
        nc, in_maps, core_ids=list(range(NCORES)), trace=trace
    )

    out = np.zeros((B, NPASS), np.float32)
    for c in range(NCORES):
        vec = np.asarray(res.results[c]["scores"]).reshape(M)
        for j, b in enumerate(assign[c]):
            out[b, :] = vec[SEGS * j + 1 : SEGS * (j + 1)]
    out *= (plen > 0).astype(np.float32)
    return out, res


def _prepare(queries, qlen, docs, plen, emb, W, bias):
    """Host-side planning: batch->core assignment, token streams, and the 8
    per-core input maps (all index/weight metadata precomputed)."""
    np_dt = _np_dt()
    np_wdt = _np_wdt()
    mseg = MPAD if DTYPE == "fp8" else M
    assign, streams = _plan(queries, qlen, docs, plen)

    # Dedup repeated token ids within a core's stream: gather each distinct
    # row once; its weight-matrix entry becomes the occurrence count per
    # segment (small integers -> exact in bf16/fp8).
    dstreams = []
    for ids, segs in streams:
        keep = segs >= 0
        ids, segs = ids[keep], segs[keep]
        uids, inv = np.unique(ids, return_inverse=True)
        wcnt = np.zeros((len(uids), M), np.float32)
        np.add.at(wcnt, (inv, segs), 1.0)
        dstreams.append((uids, wcnt))

    buckets = []
    for uids, wcnt in dstreams:
        mA = uids < SPLIT
        buckets.append(((uids[mA], wcnt[mA]), (uids[~mA] - SPLIT, wcnt[~mA])))
    TA = max(1, max(-(-len(a[0]) // C) for a, _ in buckets))
    TB = max(1, max(-(-len(b[0]) // C) for _, b in buckets))
    T = TA + TB
    TSUB = T * SUB

    # fp8 needs a power-of-two scale to lift emb out of the subnormal range;
    # it is divided back out via the per-segment reciprocals (f32).
    scale = 1.0
    if DTYPE == "fp8":
        amax = float(np.abs(emb).max())
        scale = float(2.0 ** np.floor(np.log2(400.0 / amax)))

    emb_c = np.ascontiguousarray((emb * scale).astype(np_dt) if scale != 1.0 else emb.astype(np_dt))
    wt_host = np.ascontiguousarray(W.T.reshape(16, 128, H).transpose(1, 0, 2).astype(np_wdt))
    iden = np.eye(128, dtype=np.float32)
    ones1 = np.ones((1, M), np.float32)
    selt = np.zeros((M, M), np.float32)
    for j in range(BPC):
        selt[SEGS * j, SEGS * j : SEGS * (j + 1)] = 1.0
    beta = bias.reshape(1, H).astype(np.float32)

    in_maps = []
    for c in range(NCORES):
        (aids, awc), (bids, bwc) = buckets[c]
        sid = np.zeros(T * C, np.int64)
        w = np.zeros((T * C, mseg), np.float32)
        sid[: len(aids)] = aids
        w[: len(aids), :M] = awc
        sid[TA * C : TA * C + len(bids)] = bids
        w[TA * C : TA * C + len(bids), :M] = bwc

        i16 = sid.astype(np.int16)
        idx_host = np.ascontiguousarray(np.tile(i16.reshape(-1, 16).T, (8, 1)))

        wts_host = np.ascontiguousarray(
            w.reshape(TSUB, 128, mseg).transpose(1, 0, 2).astype(np_dt)
        )

        recip = np.zeros((M, 1), np.float32)
        for j, b in enumerate(assign[c]):
            recip[SEGS * j, 0] = 1.0 / (float(qlen[b]) * scale)
            for p in range(NPASS):
                recip[SEGS * j + 1 + p, 0] = 1.0 / (float(max(int(plen[b, p]), 1)) * scale)

        in_maps.append(
            dict(
                emb=emb_c,
                wt=wt_host,
                wts=wts_host,
                idx=idx_host,
                recip=recip,
                beta=beta,
                identity=iden,
                ones1=ones1,
                selt=selt,
            )
        )
    return TA, TB, assign, in_maps


# revision 12
# speedup vs baseline: 1.0538x; 1.0340x over previous
"""Trainium2 Bass kernel for nn_LinearSelector (ragged passage scoring).

Math (per batch b):
  q_rep  = masked-mean of emb[queries[b]]            -> [D]
  p_rep  = per-passage mean of emb[docs[b]] segments -> [P, D]
  q_enc  = q_rep @ W.T + bias ; p_enc = p_rep @ W.T + bias
  scores[b, p] = q_enc . p_enc[p]   (masked by passage_lengths > 0)

Device strategy (data-parallel over batch, 4 batches/core on 8 cores):
  - One token stream per core: 32 query tokens + all real doc tokens of its
    4 batches, bucketed by vocab half (dma_gather indices are int16).
  - dma_gather pulls 512 embedding rows per call into SBUF [128, 4, D].
  - Segment sums via PE matmul with a host-built 0/1 weight matrix
    (lhsT [128 tokens, 68 segs]) accumulating into PSUM [68, D].
  - reps /= length (per-partition scalar), transpose via PE to [D, 68],
    enc = reps @ W.T + bias (W.T streamed from HBM, host-pretransposed),
    scores = rowwise dot of enc with its batch's query row (selector matmul
    + elementwise mul + free-axis reduce).
  - Host gathers the per-core [68] score vectors back to [32, 16].
"""

import sys

if "/opt/trn_rl_repo" not in sys.path:
    sys.path.insert(0, "/opt/trn_rl_repo")

import numpy as np
import ml_dtypes

B, LQ, DLEN, NPASS = 32, 32, 4096, 16
V, D, H = 50257, 2048, 2048
NCORES = 8
BPC = B // NCORES            # batches per core
SEGS = NPASS + 1             # query + 16 passages
M = BPC * SEGS               # 68 segments per core
C = 512                      # tokens per dma_gather call
SUB = C // 128               # 128-token subtiles per call
SPLIT = 32768                # int16 vocab bucket split
DTYPE = "bf16"                # "fp8" | "bf16" | "f32" for emb/gather/segment-sum
MPAD = 80                    # lhsT seg-dim padded so DoubleRow AP step % 16 == 0

_prog_cache: dict = {}


def _np_dt():
    return {"fp8": ml_dtypes.float8_e4m3, "bf16": ml_dtypes.bfloat16, "f32": np.float32}[DTYPE]


def _np_wdt():
    return ml_dtypes.bfloat16 if DTYPE != "f32" else np.float32


def _build_program(TA: int, TB: int):
    import concourse.tile as tile
    from concourse import bacc, mybir

    dt = {"fp8": mybir.dt.float8e4, "bf16": mybir.dt.bfloat16, "f32": mybir.dt.float32}[DTYPE]
    wdt = mybir.dt.bfloat16 if DTYPE != "f32" else mybir.dt.float32
    f32 = mybir.dt.float32
    fp8 = DTYPE == "fp8"
    mseg = MPAD if fp8 else M
    T = TA + TB
    TSUB = T * SUB

    nc = bacc.Bacc(
        "TRN2",
        target_bir_lowering=False,
        debug=False,
        enable_asserts=False,
        num_swdge_queues=4,
        dynamic_dma_scratch_size=16384,
    )
    emb_t = nc.dram_tensor("emb", [V, D], dt, kind="ExternalInput")
    wt_t = nc.dram_tensor("wt", [128, 16, H], wdt, kind="ExternalInput")
    wts_t = nc.dram_tensor("wts", [128, TSUB, mseg], dt, kind="ExternalInput")
    idx_t = nc.dram_tensor("idx", [128, T * C // 16], mybir.dt.int16, kind="ExternalInput")
    recip_t = nc.dram_tensor("recip", [M, 1], f32, kind="ExternalInput")
    beta_t = nc.dram_tensor("beta", [1, H], f32, kind="ExternalInput")
    iden_t = nc.dram_tensor("identity", [128, 128], f32, kind="ExternalInput")
    ones_t = nc.dram_tensor("ones1", [1, M], f32, kind="ExternalInput")
    selt_t = nc.dram_tensor("selt", [M, M], f32, kind="ExternalInput")
    scores_t = nc.dram_tensor("scores", [M, 1], f32, kind="ExternalOutput")

    with tile.TileContext(nc) as tc:
        with (
            tc.tile_pool(name="const", bufs=1) as const,
            tc.tile_pool(name="gather", bufs=5) as gather,
            tc.tile_pool(name="wtp", bufs=2) as wtp,
        ):
            idx_sb = const.tile([128, T * C // 16], mybir.dt.int16)
            wts_sb = const.tile([128, TSUB, mseg], dt)

            reps_sb = const.tile([128, D], f32)
            # rows >= M are only consumed by the PE transpose (whose M-trailing
            # output columns are discarded); zero them from partition 64 (DVE
            # base-partition must be a multiple of 32) — rows 64..M-1 are
            # rewritten by the divide below before any read.
            nc.vector.memset(reps_sb[64:128, :], 0.0)

            # --- gather + segment-sum accumulation ---
            with tc.tile_pool(name="psum_reps", bufs=1, space="PSUM") as pr:
                reps_ps = [pr.tile([mseg, 512], f32, tag=f"r{n}", name=f"reps_ps{n}") for n in range(4)]
                for g in range(T):
                    src = emb_t[0:SPLIT, :] if g < TA else emb_t[SPLIT:V, :]
                    # per-call metadata slices: keeps the first gather/matmuls
                    # from waiting on one monolithic const DMA at startup
                    nc.sync.dma_start(
                        idx_sb[:, g * (C // 16) : (g + 1) * (C // 16)],
                        idx_t[:, g * (C // 16) : (g + 1) * (C // 16)],
                    )
                    nc.sync.dma_start(
                        wts_sb[:, g * SUB : (g + 1) * SUB, :],
                        wts_t[:, g * SUB : (g + 1) * SUB, :],
                    )
                    buf = gather.tile([128, SUB, D], dt, tag="gbuf", name=f"gbuf{g}")
                    nc.gpsimd.dma_gather(
                        buf[:],
                        src,
                        idx_sb[:, g * (C // 16) : (g + 1) * (C // 16)],
                        C,
                        C,
                        D,
                        queue_num=g % 4,
                    )
                    if fp8:
                        for s in range(0, SUB, 2):
                            tt = g * SUB + s
                            for n in range(4):
                                nc.tensor.matmul(
                                    reps_ps[n][:, :],
                                    wts_sb[:, tt : tt + 2, :],
                                    buf[:, s : s + 2, n * 512 : (n + 1) * 512],
                                    start=(tt == 0),
                                    stop=(tt == TSUB - 2),
                                    perf_mode=mybir.MatmulPerfMode.DoubleRow,
                                )
                    else:
                        for s in range(SUB):
                            tt = g * SUB + s
                            for n in range(4):
                                nc.tensor.matmul(
                                    reps_ps[n][:, :],
                                    wts_sb[:, tt, :],
                                    buf[:, s, n * 512 : (n + 1) * 512],
                                    start=(tt == 0),
                                    stop=(tt == TSUB - 1),
                                )
                # reps = segment sums / lengths
                recip_sb = const.tile([M, 1], f32)
                nc.sync.dma_start(recip_sb[:], recip_t[:, :])
                for n in range(4):
                    nc.vector.tensor_scalar_mul(
                        reps_sb[:M, n * 512 : (n + 1) * 512],
                        reps_ps[n][:M, :],
                        recip_sb[:, :],
                    )

            # --- transpose reps -> repsT tiles [128, 16, M] ---
            iden_sb = const.tile([128, 128], f32)
            nc.sync.dma_start(iden_sb[:], iden_t[:, :])
            beta_sb = const.tile([1, H], f32)
            nc.sync.dma_start(beta_sb[:], beta_t[:, :])
            ones_sb = const.tile([1, M], f32)
            nc.sync.dma_start(ones_sb[:], ones_t[:, :])
            selt_sb = const.tile([M, M], f32)
            nc.sync.dma_start(selt_sb[:], selt_t[:, :])
            repsT = const.tile([128, 16, M], wdt)
            with tc.tile_pool(name="psum_tr", bufs=2, space="PSUM") as pt:
                for k in range(16):
                    trp = pt.tile([128, 128], f32, tag="tr", name=f"trp{k}")
                    nc.tensor.transpose(
                        trp[:], reps_sb[:, k * 128 : (k + 1) * 128], iden_sb[:]
                    )
                    nc.vector.tensor_copy(repsT[:, k, :], trp[:, :M])

            # --- enc = bias + reps @ W.T (W.T streamed) ---
            enc_sb = const.tile([M, H], f32)
            with tc.tile_pool(name="psum_enc", bufs=1, space="PSUM") as pe:
                enc_ps = [pe.tile([M, 512], f32, tag=f"e{n}", name=f"enc_ps{n}") for n in range(4)]
                for n in range(4):
                    nc.tensor.matmul(
                        enc_ps[n][:, :],
                        ones_sb[:, :],
                        beta_sb[:, n * 512 : (n + 1) * 512],
                        start=True,
                        stop=False,
                    )
                for kg in range(4):
                    slab = wtp.tile([128, 4, H], wdt, tag="wslab", name=f"wslab{kg}")
                    nc.sync.dma_start(slab[:], wt_t[:, kg * 4 : (kg + 1) * 4, :])
                    for k2 in range(4):
                        k = kg * 4 + k2
                        for n in range(4):
                            nc.tensor.matmul(
                                enc_ps[n][:, :],
                                repsT[:, k, :],
                                slab[:, k2, n * 512 : (n + 1) * 512],
                                start=False,
                                stop=(k == 15),
                            )
                for n in range(4):
                    nc.vector.tensor_copy(
                        enc_sb[:, n * 512 : (n + 1) * 512], enc_ps[n][:, :]
                    )

            # --- scores: row r of enc dotted with its batch's query row ---
            prod = const.tile([M, H], f32)
            with tc.tile_pool(name="psum_qx", bufs=1, space="PSUM") as pq:
                for n in range(4):
                    qx_ps = pq.tile([M, 512], f32, tag=f"q{n}", name=f"qx_ps{n}")
                    nc.tensor.matmul(
                        qx_ps[:, :],
                        selt_sb[:, :],
                        enc_sb[:, n * 512 : (n + 1) * 512],
                        start=True,
                        stop=True,
                    )
                    nc.vector.tensor_mul(
                        prod[:, n * 512 : (n + 1) * 512],
                        enc_sb[:, n * 512 : (n + 1) * 512],
                        qx_ps[:, :],
                    )
            sc = const.tile([M, 1], f32)
            nc.vector.reduce_sum(sc[:], prod[:], axis=mybir.AxisListType.X)
            nc.sync.dma_start(scores_t[:, :], sc[:])

    nc.compile()
    return nc


def _plan(queries, qlen, docs, plen):
    """Assign batches to cores (greedy-balanced) and build per-core token
    streams: (ids, segs) with seg = 17*local_b + (0 for query, 1+p for
    passage p), seg == -1 for masked/padding tokens."""
    cum = np.cumsum(plen, axis=1)
    total = cum[:, -1]
    counts = LQ + total
    order = np.argsort(-counts, kind="stable")
    loads = [0] * NCORES
    assign: list[list[int]] = [[] for _ in range(NCORES)]
    for b in order:
        cands = [c for c in range(NCORES) if len(assign[c]) < BPC]
        c = min(cands, key=lambda c: loads[c])
        assign[c].append(int(b))
        loads[c] += int(counts[b])

    streams = []
    for c in range(NCORES):
        ids_l, seg_l = [], []
        for j, b in enumerate(assign[c]):
            ids_l.append(queries[b].astype(np.int64))
            seg_l.append(np.where(np.arange(LQ) < qlen[b], SEGS * j, -1))
            nb = int(total[b])
            ids_l.append(docs[b, :nb].astype(np.int64))
            pid = np.searchsorted(cum[b], np.arange(nb), side="right")
            seg_l.append(SEGS * j + 1 + pid)
        streams.append((np.concatenate(ids_l), np.concatenate(seg_l)))
    return assign, streams


def kernel(**inputs) -> np.ndarray:
    queries = np.asarray(inputs["queries"]).astype(np.int64)
    qlen = np.asarray(inputs["query_lengths"]).astype(np.int64)
    docs = np.asarray(inputs["docs"]).astype(np.int64)
    plen = np.asarray(inputs["passage_lengths"]).astype(np.int64)
    emb = np.asarray(inputs["emb"], dtype=np.float32)
    W = np.asarray(inputs["W"], dtype=np.float32)
    bias = np.asarray(inputs["b"], dtype=np.float32)

    outputs, _ = _run(queries, qlen, docs, plen, emb, W, bias, trace=False)
    return outputs


def _run(queries, qlen, docs, plen, emb, W, bias, trace=False):
    TA, TB, assign, in_maps = _prepare(queries, qlen, docs, plen, emb, W, bias)

    key = (TA, TB, DTYPE)
    if key not in _prog_cache:
        _prog_cache[key] = _build_program(TA, TB)
    nc = _prog_cache[key]

    from concourse.bass_utils import run_bass_kernel_spmd

    res = run_bass_kernel_spmd(
        nc, in_maps, core_ids=list(range(NCORES)), trace=trace
    )

    out = np.zeros((B, NPASS), np.float32)
    for c in range(NCORES):
        vec = np.asarray(res.results[c]["scores"]).reshape(M)
        for j, b in enumerate(assign[c]):
            out[b, :] = vec[SEGS * j + 1 : SEGS * (j + 1)]
    out *= (plen > 0).astype(np.float32)
    return out, res


def _prepare(queries, qlen, docs, plen, emb, W, bias):
    """Host-side planning: batch->core assignment, token streams, and the 8
    per-core input maps (all index/weight metadata precomputed)."""
    np_dt = _np_dt()
    np_wdt = _np_wdt()
    mseg = MPAD if DTYPE == "fp8" else M
    assign, streams = _plan(queries, qlen, docs, plen)

    # Dedup repeated token ids within a core's stream: gather each distinct
    # row once; its weight-matrix entry becomes the occurrence count per
    # segment (small integers -> exact in bf16/fp8).
    dstreams = []
    for ids, segs in streams:
        keep = segs >= 0
        ids, segs = ids[keep], segs[keep]
        uids, inv = np.unique(ids, return_inverse=True)
        wcnt = np.zeros((len(uids), M), np.float32)
        np.add.at(wcnt, (inv, segs), 1.0)
        dstreams.append((uids, wcnt))

    buckets = []
    for uids, wcnt in dstreams:
        mA = uids < SPLIT
        buckets.append(((uids[mA], wcnt[mA]), (uids[~mA] - SPLIT, wcnt[~mA])))
    TA = max(1, max(-(-len(a[0]) // C) for a, _ in buckets))
    TB = max(1, max(-(-len(b[0]) // C) for _, b in buckets))
    T = TA + TB
    TSUB = T * SUB

    # fp8 needs a power-of-two scale to lift emb out of the subnormal range;
    # it is divided back out via the per-segment reciprocals (f32).
    scale = 1.0
    if DTYPE == "fp8":
        amax = float(np.abs(emb).max())
        scale = float(2.0 ** np.floor(np.log2(400.0 / amax)))

    emb_c = np.ascontiguousarray((emb * scale).astype(np_dt) if scale != 1.0 else emb.astype(np_dt))
    wt_host = np.ascontiguousarray(W.T.reshape(16, 128, H).transpose(1, 0, 2).astype(np_wdt))
    iden = np.eye(128, dtype=np.float32)
    ones1 = np.ones((1, M), np.float32)
    selt = np.zeros((M, M), np.float32)
    for j in range(BPC):
        selt[SEGS * j, SEGS * j : SEGS * (j + 1)] = 1.0
    beta = bias.reshape(1, H).astype(np.float32)

    in_maps = []
    for c in range(NCORES):
        (aids, awc), (bids, bwc) = buckets[c]
        sid = np.zeros(T * C, np.int64)
        w = np.zeros((T * C, mseg), np.float32)
        sid[: len(aids)] = aids
        w[: len(aids), :M] = awc
        sid[TA * C : TA * C + len(bids)] = bids
        w[TA * C : TA * C + len(bids), :M] = bwc

        i16 = sid.astype(np.int16)
        idx_host = np.ascontiguousarray(np.tile(i16.reshape(-1, 16).T, (8, 1)))

        wts_host = np.ascontiguousarray(
            w.reshape(TSUB, 128, mseg).transpose(1, 0, 2).astype(np_dt)
        )

        recip = np.zeros((M, 1), np.float32)
        for j, b in enumerate(assign[c]):
            recip[SEGS * j, 0] = 1.0 / (float(qlen[b]) * scale)
            for p in range(NPASS):
                recip[SEGS * j + 1 + p, 0] = 1.0 / (float(max(int(plen[b, p]), 1)) * scale)

        in_maps.append(
            dict(
                emb=emb_c,
                wt=wt_host,
                wts=wts_host,
                idx=idx_host,
                recip=recip,
                beta=beta,
                identity=iden,
                ones1=ones1,
                selt=selt,
            )
        )
    return TA, TB, assign, in_maps
